# revision 15
# baseline (speedup 1.0000x reference)
"""Causal self-attention (B=2, T=2048, D=2048, H=16, RoPE + q_gain) on 8
Trainium2 NeuronCores.

Sharding: batch (2-way) x head-groups (4-way, 4 heads each) -> 8 cores.
Each core computes the qkv projection for its 4 heads, causal attention,
and a partial output projection (Wproj row-sharded); partials are summed
on host.

Instruction-count-minimal design: ALL matmuls f32r (single-instruction,
no Ldweights) including the score matmuls — f32r q/k fit in SBUF because
the yt pool is scoped to phases 2-3; wide [128, 2048] PSUM tiles so
RoPE / exp / evacuations run one wide op instead of four narrow ones;
the softmax denominator comes from a pairwise tree reduction over the
persistent exp tile plus one ones-matmul per q-block; V is transposed
via a DRAM round-trip; x / Wqkv / Wproj ship f32r on the wire (plain
DMAs, no on-device gpsimd cast).

Runner: the compiled executable is cached across kernel() calls
(compile-once-run-many). run_bass_kernel_spmd builds a fresh
jit(shard_map(...)) closure per call, which defeats the pjit cache and
re-runs BIR verify + walrus codegen (~45us/BIR-instruction of host CPU)
on every invocation — that recompile, not device execution, was ~99.5%
of the originally reported 137ms "HW exec time". With the executable
cached and inputs device-resident, the 1x-vs-Nx wall difference
isolates true per-iteration device execution (~0.6 ms).
"""

import math

import numpy as np

import concourse.bass as bass
import concourse.mybir as mybir
from concourse import bacc
from concourse.bass_utils import run_bass_kernel_spmd
from concourse.tile import TileContext

dt = mybir.dt
F32 = dt.float32
F32R = dt.float32r
F16 = dt.float16
AF = mybir.ActivationFunctionType

D_MODEL = 2048
N_HEADS = 16
D_HEAD = 128
B = 2
T = 2048
N_CORES = 8
HG = 4            # heads per core
TB = 512          # t-block (matmul free dim)
NTB = T // TB     # 4
NCB = D_MODEL // 128   # 16 contraction tiles
NO = 3 * HG       # 12 output 128-row slices (q0..3 k0..3 v0..3)


def _emit_iteration(nc, tc, it, tensors, consts):
    (xh, wh, wph, out, dpool, qk_pool, masks) = tensors
    (cc_sb, ss_sb, gains_sb, ones_sb, onesr_sb) = consts

    v_dram = dpool.tile([HG * D_HEAD, T], F16, tag="v", name=f"v{it}")

    # q/k stored f32r [128, T] per head (fits because the yt pool is
    # scoped to phases 2-3); written by the RoPE evacuation. f32r score
    # matmuls are single-instruction (no Ldweights) unlike f16.
    q_sb = [qk_pool.tile([128, T], F32R, tag=f"q{h}", name=f"q{it}_{h}")
            for h in range(HG)]
    k_sb = [qk_pool.tile([128, T], F32R, tag=f"k{h}", name=f"k{it}_{h}")
            for h in range(HG)]
    # ---------------- Phase 1: QKV projection + RoPE ----------------
    with tc.tile_pool(name="wp1", bufs=1) as wpool, \
         tc.tile_pool(name="xp1", bufs=1) as xpool, \
         tc.tile_pool(name="st1", bufs=2) as st1, \
         tc.tile_pool(name="ps1", bufs=3, space="PSUM") as ps1, \
         tc.tile_pool(name="qs1", bufs=2, space="PSUM") as qs1:
        w_sb = wpool.tile([128, NCB * 3 * HG * 128], F32R, tag="w",
                          name=f"w{it}")
        # wh is host-prefolded to [128, (cb, o)] f32r; plain 2D copy
        nc.scalar.dma_start(out=w_sb[:], in_=wh[:])

        for tb in range(NTB):
            tsl = slice(tb * TB, (tb + 1) * TB)
            x_tb = xpool.tile([128, NCB * TB], F32R, tag="x",
                              name=f"x{it}_{tb}")
            # xh is host-prefolded to [128, (cb, t)] f32r
            nc.sync.dma_start(
                out=x_tb[:].rearrange("p (a t) -> p a t", a=NCB),
                in_=xh[:].rearrange("p (a t) -> p a t", a=NCB)[:, :, tsl])
            for o in range(NO):
                ps = ps1.tile([128, TB], F32, tag="ps", name="ps")
                for cb in range(NCB):
                    nc.tensor.matmul(
                        ps[:],
                        w_sb[:, (cb * NO + o) * 128:(cb * NO + o + 1) * 128],
                        x_tb[:, cb * TB:(cb + 1) * TB],
                        start=(cb == 0), stop=(cb == NCB - 1))
                if o < 2 * HG:
                    # RoPE evacuation:
                    #   dst[0:64]  = ps[0:64]*cc[0:64]  - ps[64:]*ss[64:]
                    #   dst[64:]   = ps[64:]*cc[64:]    + ps[0:64]*ss[0:64]
                    dst = (q_sb[o] if o < HG else k_sb[o - HG])[:, tsl]
                    nc.vector.tensor_mul(dst, ps[:], cc_sb[:, tsl])
                    qs = qs1.tile([128, TB], F32, tag="qs", name="qs")
                    nc.vector.tensor_mul(qs[:], ps[:], ss_sb[:, tsl])
                    nc.vector.tensor_sub(dst[0:64], dst[0:64], qs[64:128, :])
                    nc.vector.tensor_add(dst[64:128], dst[64:128], qs[0:64, :])
                else:
                    vs = st1.tile([128, TB], F16, tag="stage", name="vs")
                    nc.scalar.copy(vs[:], ps[:])
                    d0 = (o - 2 * HG) * 128
                    nc.sync.dma_start(out=v_dram[d0:d0 + 128, tsl], in_=vs[:])

    # ---------------- Phase 2: attention ----------------
    if "p2" in ABLATE:
        return
    # yt pool is scoped to phases 2-3 so its 32KB/partition doesn't
    # coexist with the phase-1 weight + x tiles (frees room for f32r q/k).
    ytcm = tc.tile_pool(name="ytp", bufs=1)
    yt_pool = ytcm.__enter__()
    with tc.tile_pool(name="vt2", bufs=1) as vt2, \
         tc.tile_pool(name="ep2", bufs=1) as ep2, \
         tc.tile_pool(name="sm2", bufs=2) as sm2, \
         tc.tile_pool(name="psS", bufs=1, space="PSUM") as psS, \
         tc.tile_pool(name="psY", bufs=1, space="PSUM") as psY, \
         tc.tile_pool(name="psD", bufs=1, space="PSUM") as psD, \
         tc.tile_pool(name="psR", bufs=1, space="PSUM") as psR:
        masks_sb = vt2.tile([128, 4 * TB], F32R, tag="masks",
                            name=f"mk{it}")
        nc.scalar.dma_start(out=masks_sb[:], in_=masks[:])
        yt_sb = [yt_pool.tile([128, T], F32R, tag=f"y{h}", name=f"y{it}_{h}")
                 for h in range(HG)]
        vt_sb = vt2.tile([128, NCB * HG * D_HEAD], F32R, tag="vt",
                         name=f"vt{it}")
        # vT via the XBAR DMA transpose (fp16), then cast to f32r
        with tc.tile_pool(name="vt16", bufs=1) as vt16p:
            vt16 = vt16p.tile([128, NCB * HG * D_HEAD], F16, tag="vt16",
                              name=f"vt16_{it}")
            nc.sync.dma_start_transpose(
                out=vt16[:].rearrange("p (a d) -> p a d", a=NCB),
                in_=v_dram[:])
            for j in range(4):
                jsl = slice(j * 2048, (j + 1) * 2048)
                nc.scalar.copy(vt_sb[:, jsl], vt16[:, jsl])

        for h in range(HG):
            for qb in range(NTB):
                qsl = slice(qb * TB, (qb + 1) * TB)
                y_ps = psY.tile([128, TB], F32, tag="y", name="y_ps")
                e_all = ep2.tile([128, 4 * NTB * TB], F32R, tag="eall",
                                 name="e_all")
                nkt = 4 * qb + 4
                for g in range(qb + 1):
                    s_ps = psS.tile([128, 4 * TB], F32, tag="s", name="s_ps")
                    for j in range(4):
                        kt = 4 * g + j
                        nc.tensor.matmul(
                            s_ps[:, j * TB:(j + 1) * TB],
                            k_sb[h][:, kt * 128:(kt + 1) * 128],
                            q_sb[h][:, qsl], start=True, stop=True)
                    gsl = slice(g * 4 * TB, (g + 1) * 4 * TB)
                    nc.scalar.activation(e_all[:, gsl], s_ps[:], AF.Exp,
                                         scale=gains_sb[:, h:h + 1])
                    if g == qb:  # diagonal group: causal mask
                        nc.vector.tensor_mul(e_all[:, gsl], e_all[:, gsl],
                                             masks_sb[:])
                    for j in range(4):
                        kt = 4 * g + j
                        nc.tensor.matmul(
                            y_ps[:],
                            vt_sb[:, kt * 512 + h * 128:kt * 512 + (h + 1) * 128],
                            e_all[:, kt * TB:(kt + 1) * TB],
                            start=(kt == 0), stop=(kt == nkt - 1))
                # pairwise tree: sum the nkt 512-wide strips into strip 0
                s = nkt
                while s > 1:
                    if s % 2:
                        nc.vector.tensor_add(
                            e_all[:, 0:TB], e_all[:, 0:TB],
                            e_all[:, (s - 1) * TB:s * TB])
                        s -= 1
                    half = s // 2
                    nc.vector.tensor_add(
                        e_all[:, 0:half * TB], e_all[:, 0:half * TB],
                        e_all[:, half * TB:s * TB])
                    s = half
                d_ps = psD.tile([1, TB], F32, tag="d", name="d_ps")
                nc.tensor.matmul(d_ps[:], ones_sb[:], e_all[:, 0:TB],
                                 start=True, stop=True)
                rec = sm2.tile([1, TB], F32, tag="rec", name="rec")
                nc.vector.reciprocal(rec[:], d_ps[:])
                # broadcast 1/D to all partitions via a rank-1 matmul
                recb = psR.tile([128, TB], F32, tag="recb", name="recb")
                nc.tensor.matmul(recb[:], onesr_sb[:], rec[:],
                                 start=True, stop=True)
                recb_sb = sm2.tile([128, TB], F32, tag="recbs", name="recbs")
                nc.scalar.copy(recb_sb[:], recb[:])
                nc.vector.tensor_mul(yt_sb[h][:, qsl], y_ps[:], recb_sb[:])

    # ---------------- Phase 3: output projection ----------------
    if "p3" in ABLATE:
        ytcm.__exit__(None, None, None)
        return
    with tc.tile_pool(name="wp3", bufs=1) as wp3, \
         tc.tile_pool(name="os3", bufs=2) as os3, \
         tc.tile_pool(name="psP", bufs=2, space="PSUM") as psP:
        wp_sb = wp3.tile([128, HG * D_MODEL], F32R, tag="wp", name=f"wp{it}")
        # wph is host-prefolded to [128, (h, o)] f32r; plain 2D copy
        nc.scalar.dma_start(out=wp_sb[:], in_=wph[:])
        for mt in range(NCB):
            msl = slice(mt * 128, (mt + 1) * 128)
            o_ps = psP.tile([128, D_MODEL], F32, tag="p", name="o_ps")
            for ob in range(NTB):
                osl = slice(ob * TB, (ob + 1) * TB)
                for h in range(HG):
                    nc.tensor.matmul(
                        o_ps[:, osl], yt_sb[h][:, msl],
                        wp_sb[:, h * D_MODEL + ob * TB:h * D_MODEL + (ob + 1) * TB],
                        start=(h == 0), stop=(h == HG - 1))
            o_sb = os3.tile([128, D_MODEL], F16, tag="o", name="o_sb")
            nc.scalar.copy(o_sb[:], o_ps[:])
            nc.sync.dma_start(out=out[msl, :], in_=o_sb[:])
    ytcm.__exit__(None, None, None)


ABLATE = set()  # profiling: subset of {"p2","p3","xdma","rope"}


def build_program(repeat=1):
    nc = bacc.Bacc("TRN2", target_bir_lowering=False, debug=False,
                   num_devices=N_CORES)

    xh = nc.dram_tensor("xh", [128, NCB * T], F32R, kind="ExternalInput")
    wh = nc.dram_tensor("wh", [128, NCB * 3 * HG * 128], F32R, kind="ExternalInput")
    wph = nc.dram_tensor("wph", [128, HG * D_MODEL], F32R, kind="ExternalInput")
    cc = nc.dram_tensor("cc", [128, T], F16, kind="ExternalInput")
    ss = nc.dram_tensor("ss", [128, T], F16, kind="ExternalInput")
    gains = nc.dram_tensor("gains", [128, HG], F32, kind="ExternalInput")
    masks = nc.dram_tensor("masks", [128, 4 * TB], F32R, kind="ExternalInput")
    ones = nc.dram_tensor("ones", [128, 1], F32R, kind="ExternalInput")
    onesr = nc.dram_tensor("onesr", [1, 128], F32, kind="ExternalInput")
    out = nc.dram_tensor("out", [T, D_MODEL], F16, kind="ExternalOutput")

    with TileContext(nc) as tc:
        with tc.tile_pool(name="const", bufs=1) as cpool, \
             tc.tile_pool(name="qk", bufs=1) as qk_pool, \
             tc.tile_pool(name="dram", bufs=1, space="DRAM") as dpool:
            cc_sb = cpool.tile([128, T], F16, tag="cc")
            nc.scalar.dma_start(out=cc_sb[:], in_=cc[:])
            ss_sb = cpool.tile([128, T], F16, tag="ss")
            nc.scalar.dma_start(out=ss_sb[:], in_=ss[:])
            gains_sb = cpool.tile([128, HG], F32, tag="gains")
            nc.scalar.dma_start(out=gains_sb[:], in_=gains[:])
            ones_sb = cpool.tile([128, 1], F32R, tag="ones")
            nc.scalar.dma_start(out=ones_sb[:], in_=ones[:])
            onesr_sb = cpool.tile([1, 128], F32, tag="onesr")
            nc.scalar.dma_start(out=onesr_sb[:], in_=onesr[:])

            tensors = (xh, wh, wph, out, dpool, qk_pool, masks)
            consts = (cc_sb, ss_sb, gains_sb, ones_sb, onesr_sb)
            for it in range(repeat):
                _emit_iteration(nc, tc, it, tensors, consts)

    nc.compile()
    return nc


def prepare_core_inputs(x, Wqkv, Wproj, q_gain, rope_cos, rope_sin):
    x = np.asarray(x, dtype=np.float32)
    Wqkv = np.asarray(Wqkv, dtype=np.float32)
    Wproj = np.asarray(Wproj, dtype=np.float32)
    q_gain = np.asarray(q_gain, dtype=np.float32)

    cosT = np.asarray(rope_cos, dtype=np.float16).T  # [64, T]
    sinT = np.asarray(rope_sin, dtype=np.float16).T
    cc = np.ascontiguousarray(np.concatenate([cosT, cosT], axis=0))  # [128, T]
    ss = np.ascontiguousarray(np.concatenate([sinT, sinT], axis=0))

    masks = np.zeros((128, 4 * TB), dtype=np.float32)
    kk = np.arange(128)[:, None]
    qq = np.arange(TB)[None, :]
    for j in range(4):
        masks[:, j * TB:(j + 1) * TB] = ((j * 128 + kk) <= qq)

    ones = np.ones((128, 1), dtype=np.float32)
    scale = 1.0 / math.sqrt(D_HEAD)

    def fold(aT):
        # [C, N] -> device layout [128, (cb, N)]: (p, cb, n) = aT[cb*128+p, n]
        C, N = aT.shape
        return np.ascontiguousarray(
            aT.reshape(C // 128, 128, N).transpose(1, 0, 2).reshape(128, -1)
        ).astype(np.float32)

    xh_b = [fold(x[b].T) for b in range(B)]  # x[b].T is [C, T]
    wh_hg = []
    wph_hg = []
    for hg in range(HG):
        r0 = 512 * hg
        wsel = np.concatenate([
            Wqkv[r0:r0 + 512],                              # q rows
            Wqkv[D_MODEL + r0:D_MODEL + r0 + 512],          # k rows
            Wqkv[2 * D_MODEL + r0:2 * D_MODEL + r0 + 512],  # v rows
        ], axis=0)                                           # [1536, C]
        wh_hg.append(fold(np.ascontiguousarray(wsel.T)))     # [C, 1536] folded
        wph_hg.append(fold(np.ascontiguousarray(Wproj[:, r0:r0 + 512].T)))

    in_maps = []
    for c in range(N_CORES):
        b = c // HG
        hg = c % HG
        g = (q_gain[4 * hg:4 * hg + 4] * scale).astype(np.float32)
        gains = np.ascontiguousarray(np.broadcast_to(g[None, :], (128, HG)))
        in_maps.append({
            "xh": xh_b[b], "wh": wh_hg[hg], "wph": wph_hg[hg], "cc": cc,
            "ss": ss, "gains": gains, "masks": masks, "ones": ones,
            "onesr": np.ones((1, 128), dtype=np.float32),
        })
    return in_maps


_NC_CACHE = {}
_FN_CACHE = {}


def _get_nc(repeat=1):
    if repeat not in _NC_CACHE:
        _NC_CACHE[repeat] = build_program(repeat)
    return _NC_CACHE[repeat]


def _make_callable(nc):
    """Compile-once-run-many executable for nc (avoids the per-call re-jit
    that run_bass_kernel_spmd incurs: a fresh closure per call defeats the
    pjit cache and re-runs BIR verify + walrus codegen every invocation)."""
    import jax
    from jax.experimental.shard_map import shard_map
    from jax.sharding import Mesh, NamedSharding, PartitionSpec

    from concourse import bass2jax
    import concourse.mybir as mybir_

    bass2jax.install_neuronx_cc_hook()
    partition_name = (nc.partition_id_tensor.name
                      if nc.partition_id_tensor else None)
    in_names, out_names, out_avals, zero_outs = [], [], [], []
    for alloc in nc.m.functions[0].allocations:
        if not isinstance(alloc, mybir_.MemoryLocationSet):
            continue
        name = alloc.memorylocations[0].name
        if alloc.kind == "ExternalInput":
            if name != partition_name:
                in_names.append(name)
        elif alloc.kind == "ExternalOutput":
            shape = tuple(alloc.tensor_shape)
            dtype = mybir_.dt.np(alloc.dtype)
            out_names.append(name)
            out_avals.append(jax.core.ShapedArray(shape, dtype))
            zero_outs.append(np.zeros(shape, dtype))
    all_in = list(in_names) + list(out_names)
    if partition_name is not None:
        all_in.append(partition_name)

    def _body(*args):
        operands = list(args)
        if partition_name is not None:
            operands.append(bass2jax.partition_id_tensor())
        outs = bass2jax._bass_exec_p.bind(
            *operands, out_avals=tuple(out_avals),
            in_names=tuple(all_in), out_names=tuple(out_names),
            lowering_input_output_aliases=(),
            sim_require_finite=True, sim_require_nnan=True, nc=nc)
        return tuple(outs)

    devices = jax.devices()[:N_CORES]
    mesh = Mesh(np.asarray(devices), ("core",))
    n_in = len(in_names) + len(zero_outs)
    fn = jax.jit(shard_map(_body, mesh=mesh,
                           in_specs=(PartitionSpec("core"),) * n_in,
                           out_specs=(PartitionSpec("core"),) * len(out_names),
                           check_rep=False),
                 keep_unused=True)
    sharding = NamedSharding(mesh, PartitionSpec("core"))
    return fn, sharding, in_names, out_names, zero_outs


def _get_callable(repeat=1):
    if repeat not in _FN_CACHE:
        _FN_CACHE[repeat] = _make_callable(_get_nc(repeat))
    return _FN_CACHE[repeat]


def _device_args(in_maps, sharding, in_names, zero_outs):
    import jax
    dev_in = [jax.device_put(
        np.concatenate([np.asarray(in_maps[c][n]) for c in range(N_CORES)],
                       axis=0), sharding) for n in in_names]
    dev_z = [jax.device_put(
        np.zeros((N_CORES * z.shape[0], *z.shape[1:]), z.dtype), sharding)
        for z in zero_outs]
    return dev_in + dev_z


def kernel(x, Wqkv, Wproj, q_gain, rope_cos, rope_sin):
    fn, sharding, in_names, out_names, zero_outs = _get_callable(1)
    in_maps = prepare_core_inputs(x, Wqkv, Wproj, q_gain, rope_cos, rope_sin)
    for attempt in range(3):
        args = _device_args(in_maps, sharding, in_names, zero_outs)
        outs = fn(*args)
        res = np.asarray(outs[out_names.index("out")]).astype(np.float32)
        # transient axon/device corruption shows up as NaN or wildly
        # out-of-range values; softmax-averaged outputs here are O(10)
        if attempt < 2 and (not np.isfinite(res).all()
                            or np.abs(res).max() > 1e4):
            continue
        break
    res = res.reshape(N_CORES, T, D_MODEL)
    out = np.zeros((B, T, D_MODEL), dtype=np.float32)
    for c in range(N_CORES):
        out[c // HG] += res[c]
    return out

